# revision 1
# baseline (speedup 1.0000x reference)
"""Bipartite graph convolution (GCMC-style) Trainium2 kernel, 8-core SPMD.

Math (reference): per-rating masks M_r = (adj == r), r=1..5,
  out_u = relu(d_u * sum_r (M_r @ v_feat) @ W_u[r]),  d_u = 1/deg_u
  out_v = relu(d_v * sum_r (M_r.T @ u_feat) @ W_v[r]), d_v = 1/deg_v

Device formulation (per core, u-rows sharded 1024/core):
  Fold weights on host: P_r = v_feat @ W_u[r], Q_r = u_feat_shard @ W_v[r].
  Basis trick: since adj = sum_r r*M_r,
    sum_r M_r X_r = adj @ X_1 + sum_{r=2..5} M_r @ (X_r - r*X_1)
  so only 4 on-chip is_equal mask passes per orientation (adj tile itself is
  the 5th stationary operand). A 65th moving column carries per-basis
  constants (+1 for adj, -(r-1) for M_r) so PSUM col 64 accumulates the
  degree (edge count) for free.
  PE: stationary = [128u x 128v] fp16 mask/adj tile, moving = [128, 65]
  fp16 features+deg -> PSUM f32 [128, 65].
  Phase A (out_u): lhsT = adjT tiles (streamed), 8 persistent PSUM banks
  accumulate the whole u-shard; finish deg/relu on-chip.
  Phase B (out_v): lhsT = adj tiles (resident), 8 PSUM banks per v-group of
  8, partial [8192, 65] DMA'd out; host all-reduces over cores + finishes.
"""

import numpy as np
import sys

sys.path.insert(0, "/opt/trn_rl_repo")

N_U, N_V = 8192, 8192
F = 64
R = 5
N_CORES = 8
U_SH = N_U // N_CORES          # 1024 rows per core
UC = U_SH // 128               # 8 u-chunks per core
VC = N_V // 128                # 64 v-chunks
VG = 8                         # v-groups of 8 chunks (phase B)
J = F + 1                      # 64 features + degree column

_CACHE = {}


def _build():
    import concourse.bass as bass
    import concourse.bacc as bacc
    import concourse.mybir as mybir
    import concourse.tile as tile

    dt = mybir.dt
    eq = mybir.AluOpType.is_equal
    mx = mybir.AluOpType.max
    mult = mybir.AluOpType.mult
    SQ = mybir.ActivationFunctionType.Square
    RELU = mybir.ActivationFunctionType.Relu

    nc = bacc.Bacc("TRN2", target_bir_lowering=False, debug=False,
                   num_devices=N_CORES)

    adj_h = nc.dram_tensor("adj_h", [U_SH, N_V], dt.float16,
                           kind="ExternalInput").ap()
    adjt_h = nc.dram_tensor("adjt_h", [N_V, U_SH], dt.float16,
                            kind="ExternalInput").ap()
    q_mov = nc.dram_tensor("q_mov", [128, R * UC * J], dt.float16,
                           kind="ExternalInput").ap()
    p_mov = nc.dram_tensor("p_mov", [128, R * VC * J], dt.float16,
                           kind="ExternalInput").ap()
    out_u = nc.dram_tensor("out_u_part", [U_SH, F], dt.float32,
                           kind="ExternalOutput").ap()
    out_v = nc.dram_tensor("out_v_part", [N_V, J], dt.float32,
                           kind="ExternalOutput").ap()

    def gen_masks(nc, mtiles, src, W, bias_m3):
        """Basis tiles for ratings 2..5 of src [128, W]. DVE: one-hot
        r=2,3,5 into mt_d (r2|r3|r5); ACT: ramp4 = relu(a-3) (values
        {0,1,2}, exact) into mt_a in a single op. The host moving
        tensors are solved for basis {adj, M2, M3, ramp4, M5}."""
        mt_d, mt_a = mtiles
        nc.vector.tensor_scalar(mt_d[:, 0:W], src, 2.0, None, op0=eq)
        nc.vector.tensor_scalar(mt_d[:, W:2 * W], src, 3.0, None, op0=eq)
        nc.vector.tensor_scalar(mt_d[:, 2 * W:3 * W], src, 5.0, None, op0=eq)
        nc.scalar.activation(mt_a[:], src, RELU, bias=bias_m3[:, 0:1])

    with tile.TileContext(nc) as tc:
        with tc.tile_pool(name="consts", bufs=1) as cons, \
             tc.tile_pool(name="adjres", bufs=1) as adjres, \
             tc.tile_pool(name="fin", bufs=4) as fin:

            # SBUF/partition budget (192K cap): adj resident 128K + adjt
            # stream 4x2K + masks 2x(12+4+4)K + pstream 4x.7K + q 5.1K
            q_t = cons.tile([128, R * UC * J], dt.float16, tag="q")
            bias_m3 = cons.tile([128, 1], dt.float32, tag="bm3")
            nc.gpsimd.memset(bias_m3[:], -3.0)
            # warm the ACT spline table during initial DMA wait
            warm = cons.tile([128, 1], dt.float16, tag="warm")
            nc.scalar.activation(warm[:], bias_m3[:], RELU)
            zt = cons.tile([128, 4 * J], dt.float16, tag="zt")
            nc.gpsimd.memset(zt[:], 0.0)

            adj_q = [[adjres.tile([128, 2048], dt.float16,
                                  tag=f"adj{uc}_{q}", name=f"adjr{uc}_{q}")
                      for q in range(4)] for uc in range(UC)]

            def mask_tiles(pool, W, key):
                mt_d = pool.tile([128, 3 * W], dt.float16, tag="mtd",
                                 name=f"mtd{key}")
                mt_a = pool.tile([128, W], dt.float16, tag="mta",
                                 name=f"mta{key}")
                return mt_d, mt_a

            def lhsT_of(mtiles, base, W, b, i):
                mt_d, mt_a = mtiles
                if b == 0:
                    return base[:, i * 128:(i + 1) * 128]
                if b == 3:
                    return mt_a[:, i * 128:(i + 1) * 128]
                seg = {1: 0, 2: 1, 4: 2}[b]
                return mt_d[:, seg * W + i * 128:seg * W + (i + 1) * 128]

            # ---------------- Phase A: out_u ----------------
            pspA = tc.tile_pool(name="psumA", bufs=1, space="PSUM")
            psp = pspA.__enter__()
            mpoolA = tc.tile_pool(name="maskA", bufs=4)
            mpa = mpoolA.__enter__()
            adjtp = tc.tile_pool(name="adjts", bufs=6)
            adjts = adjtp.__enter__()
            ps_u = [psp.tile([128, J], dt.float32, tag=f"psu{uc}",
                             name=f"psu{uc}") for uc in range(UC)]
            # only the first column-quarter of each adj row-block loads
            # in phase A (needed by phase B's first v-group); later
            # quarters stream during phase B one v-group ahead
            adj_sched = {30 + k * 4: k * 4 for k in range(UC)}
            for vc in range(VC):
                at = adjts.tile([128, U_SH], dt.float16, tag="adjt")
                nc.sync.dma_start(at[:], adjt_h[vc * 128:(vc + 1) * 128, :])
                pt = mpa.tile([128, R * J], dt.float16, tag="pstream",
                              bufs=4)
                nc.sync.dma_start(pt[:], p_mov[:, vc * R * J:(vc + 1) * R * J])
                # spread the 16MB resident-adj load through phase A in
                # 512KB quarters so it never head-of-line-blocks streams
                k = adj_sched.get(vc)
                if k is not None:
                    uc = k // 4
                    nc.sync.dma_start(
                        adj_q[uc][0][:],
                        adj_h[uc * 128:(uc + 1) * 128, 0:2048])
                if vc == 4:
                    nc.sync.dma_start(q_t[:], q_mov[:])
                mtiles = mask_tiles(mpa, U_SH, f"a{vc}")
                gen_masks(nc, mtiles, at[:], U_SH, bias_m3)
                for uc in range(UC):
                    for b in range(R):
                        nc.tensor.matmul(
                            ps_u[uc][:], lhsT_of(mtiles, at, U_SH, b, uc),
                            pt[:, b * J:(b + 1) * J],
                            start=(vc == 0 and b == 0),
                            stop=(vc == VC - 1 and b == R - 1))
            # finish out_u: d_u = 1/max(deg,0.5); relu(d_u * x) on ACT
            for uc in range(UC):
                dtl = fin.tile([128, 1], dt.float32, tag="deg")
                nc.vector.tensor_scalar(dtl[:], ps_u[uc][:, F:F + 1], 0.5,
                                        None, op0=mx)
                rtl = fin.tile([128, 1], dt.float32, tag="rec")
                nc.vector.reciprocal(rtl[:], dtl[:])
                otl = fin.tile([128, F], dt.float32, tag="outu")
                nc.scalar.activation(otl[:], ps_u[uc][:, 0:F], RELU,
                                     scale=rtl[:, 0:1])
                nc.sync.dma_start(out_u[uc * 128:(uc + 1) * 128, :], otl[:])

            adjtp.__exit__(None, None, None)
            mpoolA.__exit__(None, None, None)
            pspA.__exit__(None, None, None)

            # ------- Phase B: out_v partial (2 accumulators per bank) -----
            pspB = tc.tile_pool(name="psumB", bufs=1, space="PSUM")
            psp = pspB.__enter__()
            mpoolB = tc.tile_pool(name="maskB", bufs=3)
            mpb = mpoolB.__enter__()
            W2 = 2048
            for vg2 in range(4):
                ps2 = [psp.tile([128, 4 * J], dt.float32, tag=f"psv{k}",
                                name=f"psv{vg2}_{k}", bufs=2)
                       for k in range(4)]
                # dummy start=True matmul zeroes all 4 slots & owns the
                # bank-wide has_written clear; real matmuls accumulate
                for k in range(4):
                    nc.tensor.matmul(ps2[k][:], q_t[:, 0:128], zt[:],
                                     start=True, stop=False,
                                     skip_group_check=True)
                for uc in range(UC):
                    if vg2 < 3:
                        nc.sync.dma_start(
                            adj_q[uc][vg2 + 1][:],
                            adj_h[uc * 128:(uc + 1) * 128,
                                  (vg2 + 1) * W2:(vg2 + 2) * W2])
                    src = adj_q[uc][vg2][:]
                    mtiles = mask_tiles(mpb, W2, f"b{vg2}_{uc}")
                    gen_masks(nc, mtiles, src, W2, bias_m3)
                    for i in range(16):
                        k, sl = i % 4, i // 4
                        for b in range(R):
                            nc.tensor.matmul(
                                ps2[k][:, sl * J:(sl + 1) * J],
                                lhsT_of(mtiles, src, W2, b, i),
                                q_t[:, (b * UC + uc) * J:
                                    (b * UC + uc + 1) * J],
                                start=False,
                                stop=(uc == UC - 1 and b == R - 1
                                      and sl == 3),
                                skip_group_check=True)
                for k in range(4):
                    ev = fin.tile([128, 4 * J], dt.float32, tag="evac",
                                  name=f"ev{vg2}_{k}")
                    nc.scalar.copy(ev[:], ps2[k][:])
                    for sl in range(4):
                        vc = vg2 * 16 + sl * 4 + k
                        nc.sync.dma_start(
                            out_v[vc * 128:(vc + 1) * 128, :],
                            ev[:, sl * J:(sl + 1) * J])
            mpoolB.__exit__(None, None, None)
            pspB.__exit__(None, None, None)

    nc.compile()
    return nc


def _host_prep(adj, u_feature, v_feature, weight_u, weight_v):
    adj = np.asarray(adj)
    u_feature = np.asarray(u_feature, dtype=np.float32)
    v_feature = np.asarray(v_feature, dtype=np.float32)
    weight_u = np.asarray(weight_u, dtype=np.float32)
    weight_v = np.asarray(weight_v, dtype=np.float32)

    adj16 = adj.astype(np.float16)

    # P_r = v_feat @ W_u[r]  (phase A moving), Q_r = u_shard @ W_v[r] (phase B)
    P = np.einsum("vf,rfo->rvo", v_feature, weight_u)      # [R, N_V, F]
    # basis transform: X^_1 = X_1 ; X^_r = X_r - r*X_1 (r=2..5)
    Pb = np.empty((R, N_V, J), np.float32)
    Pb[0, :, :F] = P[0]
    Pb[0, :, F] = 1.0
    for r in range(2, R + 1):
        Pb[r - 1, :, :F] = P[r - 1] - r * P[0]
        Pb[r - 1, :, F] = -(r - 1)
    # basis element 3 is ramp4=relu(a-3) (not M4); element 4 pairs M5 with
    # X5 + 3*X1 - 2*X4 and degree-coefficient +2 so the span is unchanged
    Pb[4, :, :F] = P[4] + 3.0 * P[0] - 2.0 * P[3]
    Pb[4, :, F] = 2.0
    # p_mov[p, (vc*R+b)*J + j] = Pb[b, vc*128+p, j]  (vc-major for streaming)
    p_mov = np.ascontiguousarray(
        Pb.reshape(R, VC, 128, J).transpose(2, 1, 0, 3).reshape(128, R * VC * J)
    ).astype(np.float16)

    in_maps = []
    for c in range(N_CORES):
        sl = slice(c * U_SH, (c + 1) * U_SH)
        Q = np.einsum("uf,rfo->ruo", u_feature[sl], weight_v)  # [R, U_SH, F]
        Qb = np.empty((R, U_SH, J), np.float32)
        Qb[0, :, :F] = Q[0]
        Qb[0, :, F] = 1.0
        for r in range(2, R + 1):
            Qb[r - 1, :, :F] = Q[r - 1] - r * Q[0]
            Qb[r - 1, :, F] = -(r - 1)
        Qb[4, :, :F] = Q[4] + 3.0 * Q[0] - 2.0 * Q[3]
        Qb[4, :, F] = 2.0
        q_mov = np.ascontiguousarray(
            Qb.reshape(R, UC, 128, J).transpose(2, 0, 1, 3)
            .reshape(128, R * UC * J)).astype(np.float16)
        a = adj16[sl]
        in_maps.append({
            "adj_h": np.ascontiguousarray(a),
            "adjt_h": np.ascontiguousarray(a.T),
            "q_mov": q_mov,
            "p_mov": p_mov,
        })
    return in_maps


def kernel(adj, u_feature, v_feature, weight_u, weight_v, _trace=False):
    from concourse import bass_utils

    if "nc" not in _CACHE:
        _CACHE["nc"] = _build()
    nc = _CACHE["nc"]

    in_maps = _host_prep(adj, u_feature, v_feature, weight_u, weight_v)
    res = bass_utils.run_bass_kernel_spmd(
        nc, in_maps, core_ids=list(range(N_CORES)), trace=_trace)
    _CACHE["last_result"] = res

    out_u = np.concatenate([res.results[c]["out_u_part"]
                            for c in range(N_CORES)], axis=0)
    acc = np.zeros((N_V, J), np.float64)
    for c in range(N_CORES):
        acc += res.results[c]["out_v_part"]
    acc = acc.astype(np.float32)
    deg_v = acc[:, F]
    d_v = np.where(deg_v > 0, 1.0 / np.maximum(deg_v, 0.5), 0.0)
    out_v = np.maximum(acc[:, :F] * d_v[:, None], 0.0).astype(np.float32)
    return out_u, out_v



# revision 6
# speedup vs baseline: 1.3256x; 1.3256x over previous
"""Bipartite graph convolution (GCMC-style) Trainium2 kernel, 8-core SPMD.

Math (reference): per-rating masks M_r = (adj == r), r=1..5,
  out_u = relu(d_u * sum_r (M_r @ v_feat) @ W_u[r]),  d_u = 1/deg_u
  out_v = relu(d_v * sum_r (M_r.T @ u_feat) @ W_v[r]), d_v = 1/deg_v

Device formulation (per core, u-rows sharded 1024/core), v2:
  Fold weights on host: P_r = v_feat @ W_u[r], Q_r = u_shard @ W_v[r].
  Step basis: {a, s2, s3, s4, s5} with s_c(a) = (a >= c); host solves
  P^_k so that sum_k g_k(a) P^_k == P_a for a in 0..5 (0 -> 0).
  The masks/steps are the MOVING matmul operand (long 512-col streams)
  and the 64-wide features are stationary; two independent M=64 matmuls
  run concurrently in the two column halves of the PE array
  (tile_position col groups), so the array is fully utilized:
    group A (psum partitions 0:64)  <- even chunk of the pair
    group B (psum partitions 64:128) <- odd chunk of the pair
  Host sums the two halves afterwards (plus cross-core all-reduce for
  out_v), applies 1/deg scaling and relu.
  Step planes are generated on-chip from the streamed adj tiles:
  DVE is_ge for most planes, ACT sigmoid(40*(a-c+0.5)) for ~1 plane per
  position to balance engine load (both give exact 0/1 in fp16).
"""

import numpy as np
import sys

sys.path.insert(0, "/opt/trn_rl_repo")

N_U, N_V = 8192, 8192
F = 64
R = 5
N_CORES = 8
U_SH = N_U // N_CORES          # 1024 rows per core
UC = U_SH // 128               # 8 u-chunks per core
VC = N_V // 128                # 64 v-chunks
WB = 2048                      # phase-B v-window width
NWIN = N_V // WB               # 4 windows

_CACHE = {}

# engine split: positions with (index % ACT_SKIP == ACT_SKIP-1) generate
# all 4 step planes on DVE; the rest do 3 on DVE + threshold-4 on ACT.
ACT_SKIP = 8


def _build():
    import concourse.bass as bass
    import concourse.bacc as bacc
    import concourse.mybir as mybir
    import concourse.tile as tile

    dt = mybir.dt
    ge = mybir.AluOpType.is_ge
    SIG = mybir.ActivationFunctionType.Sigmoid

    nc = bacc.Bacc("TRN2", target_bir_lowering=False, debug=False,
                   num_devices=N_CORES)

    adjt_h = nc.dram_tensor("adjt_h", [N_V, U_SH], dt.float16,
                            kind="ExternalInput").ap()
    adj_h = nc.dram_tensor("adj_h", [U_SH, N_V], dt.float16,
                           kind="ExternalInput").ap()
    p_hat_h = nc.dram_tensor("p_hat_h", [128, VC * R * F], dt.float16,
                             kind="ExternalInput").ap()
    q_hat_h = nc.dram_tensor("q_hat_h", [128, UC * R * F], dt.float16,
                             kind="ExternalInput").ap()
    out_ut = nc.dram_tensor("out_ut", [128, U_SH], dt.float32,
                            kind="ExternalOutput").ap()
    out_vt = nc.dram_tensor("out_vt", [128, N_V], dt.float32,
                            kind="ExternalOutput").ap()

    def gen_planes(nc, pl, src, W, all_dve, bias_s4):
        """Write step planes s2..s5 of src into quarters of pl [128, 4W].
        Threshold c=k+2 for quarter k. DVE is_ge for all but k=2, which
        goes to ACT (sigmoid step) unless all_dve."""
        for k in range(4):
            c = k + 2
            dst = pl[:, k * W:(k + 1) * W]
            if k == 2 and not all_dve:
                # sigmoid(40*(a - (c-0.5))): exact 0/1 in fp16 for ints
                nc.scalar.activation(dst, src, SIG,
                                     bias=bias_s4[:, 0:1], scale=40.0)
            else:
                nc.vector.tensor_scalar(dst, src, float(c), None, op0=ge)

    with tile.TileContext(nc) as tc:
        with tc.tile_pool(name="consts", bufs=1) as cons, \
             tc.tile_pool(name="streamA", bufs=4) as streamA, \
             tc.tile_pool(name="planesA", bufs=3) as planesA, \
             tc.tile_pool(name="streamB", bufs=4) as streamB, \
             tc.tile_pool(name="planesB", bufs=3) as planesB, \
             tc.tile_pool(name="fin", bufs=4) as fin:

            p_hat = cons.tile([128, VC * R * F], dt.float16, tag="ph")
            q_hat = cons.tile([128, UC * R * F], dt.float16, tag="qh")
            nc.sync.dma_start(p_hat[:], p_hat_h[:])
            nc.sync.dma_start(q_hat[:], q_hat_h[:])
            bias_s4 = cons.tile([128, 1], dt.float32, tag="bs4")
            nc.gpsimd.memset(bias_s4[:], -40.0 * 3.5)

            # ---------------- Phase A: out_uT ----------------
            pspA = tc.tile_pool(name="psumA", bufs=1, space="PSUM")
            psA = pspA.__enter__()
            ps_u = psA.tile([128, U_SH], dt.float32, tag="psu")
            pos = 0
            for t in range(VC // 2):
                srcs = []
                planes = []
                for j in (0, 1):
                    vc = 2 * t + j
                    at = streamA.tile([128, U_SH], dt.float16, tag="adjt",
                                      name=f"at{vc}")
                    nc.sync.dma_start(at[:],
                                      adjt_h[vc * 128:(vc + 1) * 128, :])
                    pl = planesA.tile([128, 4 * U_SH], dt.float16,
                                      tag="plA", name=f"plA{vc}")
                    all_dve = (pos % ACT_SKIP) == ACT_SKIP - 1
                    pos += 1
                    gen_planes(nc, pl, at[:], U_SH, all_dve, bias_s4)
                    srcs.append(at)
                    planes.append(pl)
                for b in range(R):
                    for n in range(U_SH // 512):
                        for j in (0, 1):
                            vc = 2 * t + j
                            if b == 0:
                                mov = srcs[j][:, n * 512:(n + 1) * 512]
                            else:
                                off = (b - 1) * U_SH + n * 512
                                mov = planes[j][:, off:off + 512]
                            nc.tensor.matmul(
                                ps_u[64 * j:64 * (j + 1),
                                     n * 512:(n + 1) * 512],
                                p_hat[:, (vc * R + b) * F:
                                      (vc * R + b + 1) * F],
                                mov,
                                start=(t == 0 and b == 0),
                                stop=(t == VC // 2 - 1 and b == R - 1),
                                skip_group_check=True)
            # evacuate out_uT (raw; host applies deg/relu/transpose)
            for n in range(U_SH // 512):
                ev = fin.tile([128, 512], dt.float32, tag="evu",
                              name=f"evu{n}")
                if n % 2 == 0:
                    nc.scalar.copy(ev[:], ps_u[:, n * 512:(n + 1) * 512])
                else:
                    nc.vector.tensor_copy(ev[:], ps_u[:, n * 512:(n + 1) * 512])
                nc.sync.dma_start(out_ut[:, n * 512:(n + 1) * 512], ev[:])
            pspA.__exit__(None, None, None)

            # ---------------- Phase B: out_vT ----------------
            pspB = tc.tile_pool(name="psumB", bufs=1, space="PSUM")
            psB = pspB.__enter__()
            pos = 0
            for w in range(NWIN):
                psv = psB.tile([128, WB], dt.float32, tag="psv", bufs=2,
                               name=f"psv{w}")
                for up in range(UC // 2):
                    srcs = []
                    planes = []
                    for j in (0, 1):
                        uc = 2 * up + j
                        ad = streamB.tile([128, WB], dt.float16, tag="adjb",
                                          name=f"ad{w}_{uc}")
                        nc.sync.dma_start(
                            ad[:], adj_h[uc * 128:(uc + 1) * 128,
                                         w * WB:(w + 1) * WB])
                        pl = planesB.tile([128, 4 * WB], dt.float16,
                                          tag="plB", name=f"plB{w}_{uc}")
                        all_dve = (pos % ACT_SKIP) == ACT_SKIP - 1
                        pos += 1
                        gen_planes(nc, pl, ad[:], WB, all_dve, bias_s4)
                        srcs.append(ad)
                        planes.append(pl)
                    for b in range(R):
                        for n in range(WB // 512):
                            for j in (0, 1):
                                uc = 2 * up + j
                                if b == 0:
                                    mov = srcs[j][:, n * 512:(n + 1) * 512]
                                else:
                                    off = (b - 1) * WB + n * 512
                                    mov = planes[j][:, off:off + 512]
                                nc.tensor.matmul(
                                    psv[64 * j:64 * (j + 1),
                                        n * 512:(n + 1) * 512],
                                    q_hat[:, (uc * R + b) * F:
                                          (uc * R + b + 1) * F],
                                    mov,
                                    start=(up == 0 and b == 0),
                                    stop=(up == UC // 2 - 1 and b == R - 1),
                                    skip_group_check=True)
                for n in range(WB // 512):
                    ev = fin.tile([128, 512], dt.float32, tag="evv",
                                  name=f"evv{w}_{n}")
                    if n % 2 == 0:
                        nc.scalar.copy(ev[:], psv[:, n * 512:(n + 1) * 512])
                    else:
                        nc.vector.tensor_copy(ev[:],
                                              psv[:, n * 512:(n + 1) * 512])
                    nc.sync.dma_start(
                        out_vt[:, w * WB + n * 512:w * WB + (n + 1) * 512],
                        ev[:])
            pspB.__exit__(None, None, None)

    nc.compile()
    return nc


def _basis_matrix():
    # rows r=1..5, cols k: [a, s2, s3, s4, s5]
    M = np.zeros((R, R))
    for r in range(1, R + 1):
        M[r - 1, 0] = r
        for k in range(1, R):
            M[r - 1, k] = 1.0 if r >= k + 1 else 0.0
    return M


def _host_prep(adj, u_feature, v_feature, weight_u, weight_v):
    adj = np.asarray(adj)
    u_feature = np.asarray(u_feature, dtype=np.float32)
    v_feature = np.asarray(v_feature, dtype=np.float32)
    weight_u = np.asarray(weight_u, dtype=np.float32)
    weight_v = np.asarray(weight_v, dtype=np.float32)

    adj16 = adj.astype(np.float16)
    Minv = np.linalg.inv(_basis_matrix())

    # P_r = v_feat @ W_u[r]; basis-transform to step basis
    P = np.einsum("vf,rfo->rvo", v_feature, weight_u)       # [R, N_V, F]
    Pb = np.tensordot(Minv, P, axes=([1], [0]))             # [R, N_V, F]
    # p_hat[p, (vc*R+b)*F + f] = Pb[b, vc*128+p, f]
    p_hat = np.ascontiguousarray(
        Pb.reshape(R, VC, 128, F).transpose(2, 1, 0, 3).reshape(128, -1)
    ).astype(np.float16)

    in_maps = []
    for c in range(N_CORES):
        sl = slice(c * U_SH, (c + 1) * U_SH)
        Q = np.einsum("uf,rfo->ruo", u_feature[sl], weight_v)  # [R, U_SH, F]
        Qb = np.tensordot(Minv, Q, axes=([1], [0]))
        q_hat = np.ascontiguousarray(
            Qb.reshape(R, UC, 128, F).transpose(2, 1, 0, 3).reshape(128, -1)
        ).astype(np.float16)
        a = adj16[sl]
        in_maps.append({
            "adj_h": np.ascontiguousarray(a),
            "adjt_h": np.ascontiguousarray(a.T),
            "p_hat_h": p_hat,
            "q_hat_h": q_hat,
        })
    return in_maps


def kernel(adj, u_feature, v_feature, weight_u, weight_v, _trace=False):
    from concourse import bass_utils

    if "nc" not in _CACHE:
        _CACHE["nc"] = _build()
    nc = _CACHE["nc"]

    adj = np.asarray(adj)
    in_maps = _host_prep(adj, u_feature, v_feature, weight_u, weight_v)
    res = bass_utils.run_bass_kernel_spmd(
        nc, in_maps, core_ids=list(range(N_CORES)), trace=_trace)
    _CACHE["last_result"] = res

    nz = adj > 0
    deg_u = nz.sum(axis=1).astype(np.float64)
    deg_v = nz.sum(axis=0).astype(np.float64)
    d_u = np.where(deg_u > 0, 1.0 / np.maximum(deg_u, 0.5), 0.0)
    d_v = np.where(deg_v > 0, 1.0 / np.maximum(deg_v, 0.5), 0.0)

    acc_u = np.concatenate(
        [(res.results[c]["out_ut"][0:64] + res.results[c]["out_ut"][64:128]).T
         for c in range(N_CORES)], axis=0)                   # [N_U, F]
    out_u = np.maximum(acc_u * d_u[:, None], 0.0).astype(np.float32)

    acc_v = np.zeros((128, N_V), np.float64)
    for c in range(N_CORES):
        acc_v += res.results[c]["out_vt"]
    acc_v = (acc_v[0:64] + acc_v[64:128]).T                  # [N_V, F]
    out_v = np.maximum(acc_v * d_v[:, None], 0.0).astype(np.float32)
    return out_u, out_v
